# revision 128
# baseline (speedup 1.0000x reference)
"""Trainium2 Bass kernel: ViT-style LSA attention (per-head learnable scale,
diagonal self-token mask), data-parallel over batch across 8 NeuronCores.

Reference computation (per batch b of 64, N=197 tokens, D=384, H=8, DH=64):
    qkv = x @ w_qkv ; split q,k,v ; per-head scale on q@k^T scores ;
    diagonal masked to -9.9e8 ; softmax ; attn @ v ; concat heads @ w_out + b.

Sharding: batch 64 -> 8 cores x 8 batches. Weights replicated. No
collectives; host concatenates the per-core outputs.

Key structure (v2):
  * x is transposed AND split into fp8-e4m3 (hi, lo) pairs on the host
    (x*16 = hi + lo with hi = e4m3(x*16)); w_qkv likewise (w*64 = hi + lo).
    Projections run as DoubleRow fp8 matmuls pairing contraction tiles:
    x@w = x_hi@w_hi + x_lo@w_hi + x@w_lo-k2... (lo*lo mostly dropped).  The
    ~10 tile-products over the 3 k-tiles pack into 5 half-cost DR matmuls =
    2.5 bf16-equivalents per 3 k-tiles, with BETTER-than-bf16 accuracy
    (operand errors ~2^-8 relative).  The 2^-20 operand pre-scale is undone
    by a 2^-10 scale folded into each PSUM evacuation.  The per-pair x tiles
    are padded to 512 columns: the Ldweights ISA rejects stationary APs
    whose non-inner strides are not power-of-two-ish (394/1576 fail, 512
    passes), and within one PSUM accumulation start/stop pair both matmuls
    must share a stationary partition base.
  * Scores S^T[j,i] per head stay bf16 (contraction 64 can't DoubleRow);
    head quads (0-3)/(4-7) so quad 0 needs only the first half of the q/k
    evacuations; slot order (h, h+2, h+1, h+3) keeps each PSUM pair on one
    partition base.  exp on Scalar; diagonal mask applied band-only on DVE.
  * attn@v runs in NATURAL layout: out[i, head, 0:65] with a ones-column in
    v producing the softmax denominator at column 64.  One reciprocal per
    (batch, i-tile) over free-size 8, then one DVE multiply with a stride-0
    broadcast of the reciprocals normalizes and converts to bf16.
  * The normalized natural attention tile is transposed by the DMA XBAR
    (dma_start_transpose, idle DMA hardware) into a per-batch aT [128,4,208].
    Output projection + DVE bias-add runs per batch (2 tiles).
  * GPSIMD cannot touch PSUM (BIR verifier), so Pool only handles SBUF-side
    work (ones-memsets, pad memsets) and the startup weight DMAs via SWDGE
    (bypassing the shared HWDGE); PSUM evacuations split Scalar/DVE.
  * Emission is software-pipelined: the next batch's v-projection and the
    next pair's q/k projections are emitted into the exp/mask dependency
    shadow of the current batch, and batch b's output projection is emitted
    around batch b+1's attention stages, so the in-order PE queue rarely
    head-of-line blocks.  The last two batches run a drain-friendly order
    with deferred output projections and PE-array transposes (the XBAR
    round-trip would sit on the empty critical path).

build_nc(reps=R) emits the body R times (per-rep PSUM pool scopes) so HW time
can be measured by wall-clock amplification — this container has no NTFF hook.
"""

import sys

sys.path.insert(0, "/opt/trn_rl_repo")

from contextlib import ExitStack

import ml_dtypes
import numpy as np

import concourse.bass as bass
import concourse.tile as tile
from concourse import bacc, mybir
from concourse.bass_utils import run_bass_kernel_spmd

BF16 = mybir.dt.bfloat16
F32 = mybir.dt.float32
E4 = mybir.dt.float8e4
NPBF16 = ml_dtypes.bfloat16
NPE4 = ml_dtypes.float8_e4m3
DR = mybir.MatmulPerfMode.DoubleRow

NCORES = 8
B_CORE = 8            # batches per core
N = 197               # tokens per batch
D = 384               # model dim
H = 8                 # heads
DH = 64               # head dim
INNER = H * DH        # 512
T = B_CORE * N        # 1576 tokens per core

SX = 16.0             # host pre-scale on x before fp8 split
SW = 64.0             # host pre-scale on w_qkv before fp8 split
UNSCALE = 1.0 / (SX * SW)

# per-batch key/query tiles: (offset, rows)
JTILES = [(0, 128), (128, N - 128)]
# per-batch aT slot width (197 tokens + pad for the 80-row XBAR transpose)
ATW = 208

EXP = mybir.ActivationFunctionType.Exp
# head quads: quad0 = heads 0-3 (q/k blocks 0-1), quad1 = heads 4-7
# (blocks 2-3) — quad0 only needs the first half of the q/k evacuations
QUADS = ((0, 1, 2, 3), (4, 5, 6, 7))


def build_nc(reps=1):
    nc = bacc.Bacc("TRN2", target_bir_lowering=False, debug=False)

    # x^T split-fp8, per batch pair, token dim padded to 512 so every
    # stationary AP stride is a power of two (Ldweights ISA constraint)
    xdr = nc.dram_tensor("xdr", [4, 128, 3, 2, 512], E4, kind="ExternalInput").ap()
    # w hi-parts duplicated per DR slot: [128, kt(3), s(2), 512]
    wqa = nc.dram_tensor("wqa", [128, 3, 2, INNER], E4, kind="ExternalInput").ap()
    wka = nc.dram_tensor("wka", [128, 3, 2, INNER], E4, kind="ExternalInput").ap()
    wva = nc.dram_tensor("wva", [128, 3, 2, INNER], E4, kind="ExternalInput").ap()
    # w lo-parts: [128, de(2), s(2), 512]; de=0 slots (lo-k0, lo-k1),
    # de=1 slots (ZERO, lo-k2)  — pairs with x slots (hi-k0,hi-k1)/(hi-k1,hi-k2)
    wqd = nc.dram_tensor("wqd", [128, 2, 2, INNER], E4, kind="ExternalInput").ap()
    wkd = nc.dram_tensor("wkd", [128, 2, 2, INNER], E4, kind="ExternalInput").ap()
    wvd = nc.dram_tensor("wvd", [128, 2, 2, INNER], E4, kind="ExternalInput").ap()
    # w_out rows permuted to HSLOT head order: [512, 384]
    wo = nc.dram_tensor("wo", [INNER, D], BF16, kind="ExternalInput").ap()
    # zero-diagonal band mask [128, 4, 128]
    mask = nc.dram_tensor("mask01", [128, 4, 128], BF16, kind="ExternalInput").ap()
    bias = nc.dram_tensor("bias", [128, D], F32, kind="ExternalInput").ap()
    brow = nc.dram_tensor("brow", [1, D], BF16, kind="ExternalInput").ap()
    ones = nc.dram_tensor("ones", [1, 128], BF16, kind="ExternalInput").ap()
    iddr = nc.dram_tensor("ident", [128, 128], BF16, kind="ExternalInput").ap()
    out = nc.dram_tensor("out", [T, D], F32, kind="ExternalOutput").ap()

    with tile.TileContext(nc) as tc, ExitStack() as ctx:
        const = ctx.enter_context(tc.tile_pool(name="const", bufs=1))

        xdr_sb = [
            const.tile([128, 3, 2, 512], E4, name=f"xdr{p}") for p in range(4)
        ]
        wqa_sb = const.tile([128, 3, 2, INNER], E4)
        wka_sb = const.tile([128, 3, 2, INNER], E4)
        wva_sb = const.tile([128, 3, 2, INNER], E4)
        wqd_sb = const.tile([128, 2, 2, INNER], E4)
        wkd_sb = const.tile([128, 2, 2, INNER], E4)
        wvd_sb = const.tile([128, 2, 2, INNER], E4)
        wo_sb = const.tile([128, 4, D], BF16)
        mk_sb = const.tile([128, 4, 128], BF16)
        bi_sb = const.tile([128, D], F32)
        br_sb = const.tile([1, D], BF16)
        on_sb = const.tile([1, 128], BF16)
        ident = const.tile([128, 128], BF16)

        # SBUF pools
        qt_pool = ctx.enter_context(tc.tile_pool(name="qt", bufs=3))
        kt_pool = ctx.enter_context(tc.tile_pool(name="kt", bufs=3))
        vv_pool = ctx.enter_context(tc.tile_pool(name="vv", bufs=4))
        pt_pool = ctx.enter_context(tc.tile_pool(name="pt", bufs=6))
        rp_pool = ctx.enter_context(tc.tile_pool(name="rp", bufs=6))
        an_pool = ctx.enter_context(tc.tile_pool(name="an", bufs=6))
        at_pool = ctx.enter_context(tc.tile_pool(name="at", bufs=3))
        ob_pool = ctx.enter_context(tc.tile_pool(name="ob", bufs=4))

        # PSUM: big pool (2-bank tiles, 3 bufs = 6 banks) hosts q/k-proj
        # double-groups [128,2,512], score tiles [128,4,256] and natural-
        # attention tiles [128,8,128]; small pool (1-bank, 2 bufs) hosts
        # v-proj [128,512] and out-proj [128,384] tiles.  6 + 2 = 8 banks.
        pB = ctx.enter_context(tc.tile_pool(name="pB", bufs=3, space="PSUM"))
        pS = ctx.enter_context(tc.tile_pool(name="pS", bufs=2, space="PSUM"))

        state: dict = {}

        def load_inputs(rep):
            # startup DMAs spread over all four DGE queues so issue overhead
            # and transfers overlap; each queue in consumption order
            # SP: q-weights + first-pair x chunks (split per kt)
            nc.sync.dma_start(out=wqa_sb[:], in_=wqa)
            nc.sync.dma_start(out=xdr_sb[0][:, 0], in_=xdr[0, :, 0])
            for bp in range(1, 4):
                nc.sync.dma_start(out=xdr_sb[bp][:], in_=xdr[bp])
            # Activation queue: the kt1 x-chunk in parallel with the SP
            # queue, then q lo-weights (group 0's 3rd matmul), k-weights
            nc.scalar.dma_start(out=xdr_sb[0][:, 1], in_=xdr[0, :, 1])
            nc.scalar.dma_start(out=xdr_sb[0][:, 2], in_=xdr[0, :, 2])
            nc.scalar.dma_start(out=wqd_sb[:], in_=wqd)
            nc.scalar.dma_start(out=wka_sb[:], in_=wka)
            nc.scalar.dma_start(out=wkd_sb[:], in_=wkd)
            # Pool/SWDGE queue: everything else, in consumption order
            nc.gpsimd.dma_start(out=wva_sb[:], in_=wva)
            nc.gpsimd.dma_start(out=wvd_sb[:], in_=wvd)
            nc.gpsimd.dma_start(out=mk_sb[:], in_=mask)
            nc.gpsimd.dma_start(
                out=wo_sb[:], in_=wo.rearrange("(t p) n -> p t n", p=128)
            )
            nc.gpsimd.dma_start(out=bi_sb[:], in_=bias)
            nc.gpsimd.dma_start(out=br_sb[:], in_=brow)
            nc.gpsimd.dma_start(out=on_sb[:], in_=ones)
            nc.gpsimd.dma_start(out=ident[:], in_=iddr)

        def qk_group(pair, g, eng=None):
            """One ft-pair group of the q^T/k^T projection (g in 0..3)."""
            t_p = 2 * pair * N
            if g == 0:
                state[("qk", pair)] = (
                    qt_pool.tile([128, 4, 2 * N], BF16, tag="qt", name=f"qT{pair}"),
                    kt_pool.tile([128, 4, 2 * N], BF16, tag="kt", name=f"kT{pair}"),
                )
            qT, kT = state[("qk", pair)]
            # group order (q-ft01, k-ft01, q-ft23, k-ft23): the first two
            # unblock head quad 0 of both batches
            wa_sb, wd_sb, dstT = (
                (wqa_sb, wqd_sb, qT) if g % 2 == 0 else (wka_sb, wkd_sb, kT)
            )
            fp = g // 2
            xp = xdr_sb[pair]
            ps = pB.tile([128, 2, INNER], F32, tag="pB")
            for fi in range(2):
                ft = 2 * fp + fi
                fsl = slice(ft * 128, (ft + 1) * 128)
                # kt2-dependent matmuls last: covers the kt2 chunk DMA
                for mi in range(5):
                    if mi < 2:
                        lhsT, rhs = wa_sb[:, mi, :, fsl], xp[:, mi, :, 0 : 2 * N]
                    elif mi == 2:
                        lhsT, rhs = wd_sb[:, 0, :, fsl], xp[:, 0:2, 0, 0 : 2 * N]
                    elif mi == 3:
                        lhsT, rhs = wa_sb[:, 2, :, fsl], xp[:, 2, :, 0 : 2 * N]
                    else:
                        lhsT, rhs = wd_sb[:, 1, :, fsl], xp[:, 2, :, 0 : 2 * N]
                    nc.tensor.matmul(
                        ps[:, fi, : 2 * N],
                        lhsT=lhsT,
                        rhs=rhs,
                        start=(mi == 0),
                        stop=(mi == 4),
                        perf_mode=DR,
                    )
            # one evac per ft-pair, 2^-10 unscale folded in
            if eng is None:
                eng = "s"
            dst = dstT[:, 2 * fp : 2 * fp + 2, :]
            if eng == "s":
                nc.scalar.mul(dst, ps[:, :, : 2 * N], UNSCALE)
            else:
                nc.vector.tensor_scalar_mul(dst, ps[:, :, : 2 * N], UNSCALE)

        def qk_proj(pair):
            for g in range(4):
                qk_group(pair, g)

        def qk_single(pair, w, ft, eng):
            """Single-ft q/k projection group on the small PSUM pool — used
            in the prologue to spread PSUM recycling across 5 buffers."""
            t_p = 2 * pair * N
            qT, kT = state[("qk", pair)]
            wa_sb, wd_sb, dstT = (
                (wqa_sb, wqd_sb, qT) if w == 0 else (wka_sb, wkd_sb, kT)
            )
            fsl = slice(ft * 128, (ft + 1) * 128)
            xp = xdr_sb[pair]
            ps = pS.tile([128, 2 * N], F32, tag="pS", name=f"pqs{w}_{ft}")
            for mi in range(5):
                if mi < 2:
                    lhsT, rhs = wa_sb[:, mi, :, fsl], xp[:, mi, :, 0 : 2 * N]
                elif mi == 2:
                    lhsT, rhs = wd_sb[:, 0, :, fsl], xp[:, 0:2, 0, 0 : 2 * N]
                elif mi == 3:
                    lhsT, rhs = wa_sb[:, 2, :, fsl], xp[:, 2, :, 0 : 2 * N]
                else:
                    lhsT, rhs = wd_sb[:, 1, :, fsl], xp[:, 2, :, 0 : 2 * N]
                nc.tensor.matmul(
                    ps[:, :],
                    lhsT=lhsT,
                    rhs=rhs,
                    start=(mi == 0),
                    stop=(mi == 4),
                    perf_mode=DR,
                )
            if eng == "s":
                nc.scalar.mul(dstT[:, ft, :], ps[:, :], UNSCALE)
            else:
                nc.vector.tensor_scalar_mul(dstT[:, ft, :], ps[:, :], UNSCALE)

        def v_proj(b):
            """v natural (ones column per head) for batch b, split-fp8 DR."""
            xp = xdr_sb[b // 2]
            off = (b % 2) * N
            vv = vv_pool.tile([128, 2, H * 65], BF16, tag="vv")
            for jt, (j0, jsz) in enumerate(JTILES):
                jb = slice(off + j0, off + j0 + jsz)
                pv = pS.tile([128, INNER], F32, tag="pS")
                for mi in range(5):
                    if mi < 2:
                        lhsT, rhs = xp[:, mi, :, jb], wva_sb[:, mi, :, :]
                    elif mi == 2:
                        lhsT, rhs = xp[:, 0:2, 0, jb], wvd_sb[:, 0, :, :]
                    elif mi == 3:
                        lhsT, rhs = xp[:, 2, :, jb], wva_sb[:, 2, :, :]
                    else:
                        lhsT, rhs = xp[:, 2, :, jb], wvd_sb[:, 1, :, :]
                    nc.tensor.matmul(
                        pv[:jsz, :],
                        lhsT=lhsT,
                        rhs=rhs,
                        start=(mi == 0),
                        stop=(mi == 4),
                        perf_mode=DR,
                    )
                vj = vv[:jsz, jt].rearrange("p (h c) -> p h c", c=65)
                nc.gpsimd.memset(vj[:, :, 64:65], 1.0)
                pvh = pv[:jsz, :].rearrange("p (h c) -> p h c", c=64)
                nc.vector.tensor_scalar_mul(vj[:, :, 0:64], pvh, UNSCALE)
            state[("vv", b)] = vv

        def scores(b, qi, fine=False):
            """S^T + exp + band mask for head quad qi of batch b.

            fine=True runs exp/mask in 2-head slices so downstream attn@v
            matmuls unblock sooner (used for the drain batch)."""
            qT, kT = state[("qk", b // 2)]
            off = (b % 2) * N
            # slot order pairs same-base heads (h, h+2) so each PSUM
            # accumulation pair shares one stationary partition base
            quad = QUADS[qi]
            hseq = (quad[0], quad[2], quad[1], quad[3])
            pt = pt_pool.tile([128, 2, 4, N], BF16, tag="pt")
            state[("pt", b, qi)] = pt
            for jt, (j0, jsz) in enumerate(JTILES):
                pd = pB.tile([128, 4, 256], F32, tag="pB")
                for hh, h in enumerate(hseq):
                    po = (h % 2) * 64
                    nc.tensor.matmul(
                        pd[:jsz, hh, :N],
                        lhsT=kT[po : po + 64, h // 2, off + j0 : off + j0 + jsz],
                        rhs=qT[po : po + 64, h // 2, off : off + N],
                        start=(hh % 2 == 0),
                        stop=(hh % 2 == 1),
                    )
                if fine:
                    for hp in range(2):
                        nc.scalar.activation(
                            pt[:jsz, jt, 2 * hp : 2 * hp + 2],
                            pd[:jsz, 2 * hp : 2 * hp + 2, :N],
                            EXP,
                        )
                        nc.vector.tensor_mul(
                            pt[:jsz, jt, 2 * hp : 2 * hp + 2, j0 : j0 + jsz],
                            pt[:jsz, jt, 2 * hp : 2 * hp + 2, j0 : j0 + jsz],
                            mk_sb[:jsz, 2 * hp : 2 * hp + 2, :jsz],
                        )
                else:
                    nc.scalar.activation(pt[:jsz, jt], pd[:jsz, :, :N], EXP)
                    # zero the self-token diagonal, band only (Pool: SBUF-
                    # only op, keeps the loaded DVE queue clear)
                    nc.gpsimd.tensor_mul(
                        pt[:jsz, jt, :, j0 : j0 + jsz],
                        pt[:jsz, jt, :, j0 : j0 + jsz],
                        mk_sb[:jsz, :, :jsz],
                    )

        def attnv(b, it, xbar=True, norm_pool=False):
            """attn @ [v|1] natural for i-tile `it`; normalize; transpose."""
            vv = state[("vv", b)]
            pts = (state[("pt", b, 0)], state[("pt", b, 1)])
            i0, isz = JTILES[it]
            pa = pB.tile([128, 8, 128], F32, tag="pB")
            for bank in range(2):
                for si in range(4):
                    h = 4 * bank + si
                    ptq = pts[h // 4]
                    slot = (0, 2, 1, 3)[h % 4]
                    for jt, (j0, jsz) in enumerate(JTILES):
                        nc.tensor.matmul(
                            pa[:isz, h, 0:65],
                            lhsT=ptq[:jsz, jt, slot, i0 : i0 + isz],
                            rhs=vv[:jsz, jt, h * 65 : h * 65 + 65],
                            start=(si == 0 and jt == 0),
                            stop=(si == 3 and jt == 1),
                        )
            rp = rp_pool.tile([128, 8], F32, tag="rp")
            nc.vector.reciprocal(rp[:isz, :].unsqueeze(2), pa[:isz, :, 64:65])
            an = an_pool.tile([128, INNER], BF16, tag="an")
            if it == 1 and xbar:
                # pad rows 69..80 for the XBAR transpose (base partition must
                # be 0/32/64/96; rows 64:69 are overwritten by the multiply)
                nc.gpsimd.memset(an[64:80, :], 0.0)
            norm = nc.gpsimd.tensor_mul if norm_pool else nc.vector.tensor_mul
            norm(
                an[:isz].rearrange("p (s c) -> p s c", c=64),
                pa[:isz, :, 0:64],
                rp[:isz, :].unsqueeze(2).broadcast_to([isz, 8, 64]),
            )
            if it == 0:
                aT = at_pool.tile([128, 4, ATW], BF16, tag="at")
                state[("at", b)] = aT
            else:
                aT = state[("at", b)]
            state[("an", b, it)] = an
            if xbar:
                if it == 0:
                    nc.sync.dma_start_transpose(
                        out=aT[:, :, 0:128], in_=an[0:128, :]
                    )
                else:
                    nc.sync.dma_start_transpose(
                        out=aT[:, :, 128:208], in_=an[0:80, :]
                    )

        def pe_tr(b, it):
            """PE-array transpose of an(b, it) into aT — for the drain batch,
            where the XBAR round-trip latency would sit on the critical path."""
            an = state.pop(("an", b, it))
            aT = state[("at", b)]
            rows = JTILES[it][1]
            rp_ = rows + rows % 2  # keep 4-byte slot alignment for PSUM
            psT = pS.tile([128, 4, rp_], BF16, tag="pS", name=f"psT{b}_{it}")
            for kb in range(4):
                nc.tensor.transpose(
                    psT[:, kb, :rows],
                    an[0:rows, kb * 128 : (kb + 1) * 128],
                    ident[:rows, :rows],
                )
            if it == 0:
                nc.vector.tensor_copy(aT[:, :, 0:128], psT[:, :, :rows])
            else:
                nc.scalar.copy(aT[:, :, 128 : 128 + rows], psT[:, :, :rows])

        def out_tile(b, it, tail=False, ceng="s"):
            """output projection + bias for batch b, token tile `it`.

            tail=True folds the bias in as a ones-row matmul (PE is idle in
            the drain) and evacuates with a Scalar copy, keeping the slow
            engines off the critical tail chain."""
            aT = state[("at", b)]
            t_b = b * N
            j0, jsz = JTILES[it]
            bias_mm = tail
            po_ = pS.tile([128, D], F32, tag="pS")
            for kf in range(4):
                nc.tensor.matmul(
                    po_[:jsz, :],
                    lhsT=aT[:, kf, j0 : j0 + jsz],
                    rhs=wo_sb[:, kf, :],
                    start=(kf == 0),
                    stop=(kf == 3) if not bias_mm else False,
                )
            ob = ob_pool.tile([128, D], F32, tag="ob")
            if bias_mm:
                # bias as a ones-row rank-1 matmul; evac is then a plain
                # copy, which the Scalar engine can take (it cannot add)
                nc.tensor.matmul(
                    po_[:jsz, :],
                    lhsT=on_sb[:1, :jsz],
                    rhs=br_sb[:1, :],
                    start=False,
                    stop=True,
                )
                if ceng == "s":
                    nc.scalar.copy(ob[:jsz, :], po_[:jsz, :])
                else:
                    nc.vector.tensor_copy(ob[:jsz, :], po_[:jsz, :])
            else:
                nc.vector.tensor_add(ob[:jsz, :], po_[:jsz, :], bi_sb[:jsz, :])
            nc.sync.dma_start(
                out=out[t_b + j0 : t_b + j0 + jsz, :], in_=ob[:jsz, :]
            )

        def out_proj(b, tail=False):
            out_tile(b, 0, tail=tail)
            out_tile(b, 1, tail=tail)

        for _rep in range(reps):
            if _rep == 0:
                load_inputs(_rep)
            # software-pipelined batch loop: qk groups interleave with scores
            # on odd batches; out_proj fills the exp/mask shadow on even ones
            # prologue: 5 PSUM buffers + 3 evac engines kill recycle stalls
            qk_group(0, 0, eng="s")
            qk_group(0, 1, eng="v")
            qk_group(0, 2, eng="s")
            qk_single(0, 1, 2, "s")
            qk_single(0, 1, 3, "v")
            v_proj(0)
            v_proj(1)
            for b in range(B_CORE - 2):
                scores(b, 0)
                scores(b, 1)
                if b % 2 == 0 and b // 2 + 1 <= 3:
                    # next pair's projections fill the exp/mask shadow,
                    # interleaved against their own PSUM-recycle latency
                    qk_group(b // 2 + 1, 0)
                    qk_group(b // 2 + 1, 1)
                    if 1 <= b:
                        v_proj(b + 1)
                    qk_group(b // 2 + 1, 2)
                    attnv(b, 0)
                    qk_group(b // 2 + 1, 3)
                    out_proj(b - 1) if b >= 1 else None
                else:
                    v_proj(b + 1)
                    out_proj(b - 1)
                    attnv(b, 0)
                attnv(b, 1)
            # last two batches: drain-friendly — batch 7's first score quad
            # moves early so the serial exp queue finishes sooner; deferred
            # output projections feed the PE while the final chains settle;
            # PE transposes replace the XBAR for batch 7 (the XBAR round-trip
            # would sit on the now-empty critical path)
            b = B_CORE - 2
            scores(b, 0)
            scores(b, 1)
            v_proj(b + 1)
            scores(b + 1, 0)
            out_proj(b - 1)
            attnv(b, 0)
            scores(b + 1, 1)
            attnv(b, 1)
            b = B_CORE - 1
            attnv(b, 0, xbar=False)
            out_proj(b - 1, tail=True)
            attnv(b, 1, xbar=False)
            pe_tr(b, 0)
            pe_tr(b, 1)
            out_tile(b, 0, tail=True)
            out_tile(b, 1, tail=True, ceng="v")

    return nc


_CACHE: dict = {}


def get_compiled():
    if "nc" not in _CACHE:
        nc = build_nc()
        nc.compile()
        _CACHE["nc"] = nc
    return _CACHE["nc"]


def _split8(a, s):
    a = np.asarray(a, np.float32) * s
    hi = a.astype(NPE4)
    lo = (a - hi.astype(np.float32)).astype(NPE4)
    return hi, lo


def make_in_maps(x, w_qkv, scale, w_out, b_out):
    x = np.asarray(x, np.float32)
    w_qkv = np.asarray(w_qkv, np.float32)
    scale = np.asarray(scale, np.float32)
    w_out = np.asarray(w_out, np.float32)
    b_out = np.asarray(b_out, np.float32)

    # fold the per-head LSA scale into Wq (exact in real arithmetic)
    scale_rep = np.repeat(scale, DH)  # [512]
    wq = w_qkv[:, :INNER] * scale_rep[None, :]
    wk = w_qkv[:, INNER : 2 * INNER]
    wv = w_qkv[:, 2 * INNER :]

    def w_packs(w):
        hi, lo = _split8(w, SW)  # [384, 512] each
        hi = hi.reshape(3, 128, INNER)
        lo = lo.reshape(3, 128, INNER)
        wa = np.empty((128, 3, 2, INNER), NPE4)
        wa[:, :, 0, :] = hi.transpose(1, 0, 2)
        wa[:, :, 1, :] = hi.transpose(1, 0, 2)
        wd = np.zeros((128, 2, 2, INNER), NPE4)
        wd[:, 0, 0, :] = lo[0]
        wd[:, 0, 1, :] = lo[1]
        # de=1: lo-k2 duplicated — pairs with x (hi-k2, lo-k2) to add
        # (x_hi + x_lo)*w_lo-k2, using the otherwise-empty slot
        wd[:, 1, 0, :] = lo[2]
        wd[:, 1, 1, :] = lo[2]
        return wa, wd

    wqa, wqd = w_packs(wq)
    wka, wkd = w_packs(wk)
    wva, wvd = w_packs(wv)

    wo = w_out.astype(NPBF16)

    bias = np.ascontiguousarray(np.broadcast_to(b_out, (128, D))).astype(np.float32)

    # zero-diagonal band mask, replicated along the head-quad axis
    mk = np.ones((128, 128), np.float32)
    np.fill_diagonal(mk, 0.0)
    mk = np.repeat(mk[:, None, :], 4, axis=1).astype(NPBF16)

    xs = x.reshape(NCORES, B_CORE * N, D)
    in_maps = []
    for c in range(NCORES):
        hi, lo = _split8(xs[c].T, SX)  # [384, T]
        xdr = np.zeros((4, 128, 3, 2, 512), NPE4)
        hi = hi.reshape(3, 128, 4, 2 * N)
        lo = lo.reshape(3, 128, 4, 2 * N)
        xdr[:, :, :, 0, 0 : 2 * N] = hi.transpose(2, 1, 0, 3)
        xdr[:, :, :, 1, 0 : 2 * N] = lo.transpose(2, 1, 0, 3)
        in_maps.append(
            {
                "xdr": xdr,
                "wqa": wqa,
                "wqd": wqd,
                "wka": wka,
                "wkd": wkd,
                "wva": wva,
                "wvd": wvd,
                "wo": wo,
                "mask01": mk,
                "bias": bias,
                "brow": b_out.reshape(1, D).astype(NPBF16),
                "ones": np.ones((1, 128), NPBF16),
                "ident": np.eye(128, dtype=NPBF16),
            }
        )
    return in_maps


def run(x, w_qkv, scale, w_out, b_out, trace=False):
    """Run on the 8 NeuronCores; returns (full_output, BassKernelResults)."""
    in_maps = make_in_maps(x, w_qkv, scale, w_out, b_out)
    nc = get_compiled()
    res = run_bass_kernel_spmd(nc, in_maps, core_ids=list(range(NCORES)), trace=trace)
    outs = [res.results[c]["out"].reshape(B_CORE, N, D) for c in range(NCORES)]
    full = np.concatenate(outs, axis=0).astype(np.float32)
    return full, res


def kernel(x, w_qkv, scale, w_out, b_out):
    full, _ = run(x, w_qkv, scale, w_out, b_out, trace=False)
    return full


# revision 129
# speedup vs baseline: 1.0003x; 1.0003x over previous
"""Trainium2 Bass kernel: ViT-style LSA attention (per-head learnable scale,
diagonal self-token mask), data-parallel over batch across 8 NeuronCores.

Reference computation (per batch b of 64, N=197 tokens, D=384, H=8, DH=64):
    qkv = x @ w_qkv ; split q,k,v ; per-head scale on q@k^T scores ;
    diagonal masked to -9.9e8 ; softmax ; attn @ v ; concat heads @ w_out + b.

Sharding: batch 64 -> 8 cores x 8 batches. Weights replicated. No
collectives; host concatenates the per-core outputs.

Key structure (v2):
  * x is transposed AND split into fp8-e4m3 (hi, lo) pairs on the host
    (x*16 = hi + lo with hi = e4m3(x*16)); w_qkv likewise (w*64 = hi + lo).
    Projections run as DoubleRow fp8 matmuls pairing contraction tiles:
    x@w = x_hi@w_hi + x_lo@w_hi + x@w_lo-k2... (lo*lo mostly dropped).  The
    ~10 tile-products over the 3 k-tiles pack into 5 half-cost DR matmuls =
    2.5 bf16-equivalents per 3 k-tiles, with BETTER-than-bf16 accuracy
    (operand errors ~2^-8 relative).  The 2^-20 operand pre-scale is undone
    by a 2^-10 scale folded into each PSUM evacuation.  The per-pair x tiles
    are padded to 512 columns: the Ldweights ISA rejects stationary APs
    whose non-inner strides are not power-of-two-ish (394/1576 fail, 512
    passes), and within one PSUM accumulation start/stop pair both matmuls
    must share a stationary partition base.
  * Scores S^T[j,i] per head stay bf16 (contraction 64 can't DoubleRow);
    head quads (0-3)/(4-7) so quad 0 needs only the first half of the q/k
    evacuations; slot order (h, h+2, h+1, h+3) keeps each PSUM pair on one
    partition base.  exp on Scalar; diagonal mask applied band-only on DVE.
  * attn@v runs in NATURAL layout: out[i, head, 0:65] with a ones-column in
    v producing the softmax denominator at column 64.  One reciprocal per
    (batch, i-tile) over free-size 8, then one DVE multiply with a stride-0
    broadcast of the reciprocals normalizes and converts to bf16.
  * The normalized natural attention tile is transposed by the DMA XBAR
    (dma_start_transpose, idle DMA hardware) into a per-batch aT [128,4,208].
    Output projection + DVE bias-add runs per batch (2 tiles).
  * GPSIMD cannot touch PSUM (BIR verifier), so Pool only handles SBUF-side
    work (ones-memsets, pad memsets) and the startup weight DMAs via SWDGE
    (bypassing the shared HWDGE); PSUM evacuations split Scalar/DVE.
  * Emission is software-pipelined: the next batch's v-projection and the
    next pair's q/k projections are emitted into the exp/mask dependency
    shadow of the current batch, and batch b's output projection is emitted
    around batch b+1's attention stages, so the in-order PE queue rarely
    head-of-line blocks.  The last two batches run a drain-friendly order
    with deferred output projections and PE-array transposes (the XBAR
    round-trip would sit on the empty critical path).

build_nc(reps=R) emits the body R times (per-rep PSUM pool scopes) so HW time
can be measured by wall-clock amplification — this container has no NTFF hook.
"""

import sys

sys.path.insert(0, "/opt/trn_rl_repo")

from contextlib import ExitStack

import ml_dtypes
import numpy as np

import concourse.bass as bass
import concourse.tile as tile
from concourse import bacc, mybir
from concourse.bass_utils import run_bass_kernel_spmd

BF16 = mybir.dt.bfloat16
F32 = mybir.dt.float32
E4 = mybir.dt.float8e4
NPBF16 = ml_dtypes.bfloat16
NPE4 = ml_dtypes.float8_e4m3
DR = mybir.MatmulPerfMode.DoubleRow

NCORES = 8
B_CORE = 8            # batches per core
N = 197               # tokens per batch
D = 384               # model dim
H = 8                 # heads
DH = 64               # head dim
INNER = H * DH        # 512
T = B_CORE * N        # 1576 tokens per core

SX = 16.0             # host pre-scale on x before fp8 split
SW = 64.0             # host pre-scale on w_qkv before fp8 split
UNSCALE = 1.0 / (SX * SW)

# per-batch key/query tiles: (offset, rows)
JTILES = [(0, 128), (128, N - 128)]
# per-batch aT slot width (197 tokens + pad for the 80-row XBAR transpose)
ATW = 208

EXP = mybir.ActivationFunctionType.Exp
# head quads: quad0 = heads 0-3 (q/k blocks 0-1), quad1 = heads 4-7
# (blocks 2-3) — quad0 only needs the first half of the q/k evacuations
QUADS = ((0, 1, 2, 3), (4, 5, 6, 7))


def build_nc(reps=1):
    nc = bacc.Bacc("TRN2", target_bir_lowering=False, debug=False)

    # x^T split-fp8, per batch pair, token dim padded to 512 so every
    # stationary AP stride is a power of two (Ldweights ISA constraint)
    xdr = nc.dram_tensor("xdr", [4, 128, 3, 2, 512], E4, kind="ExternalInput").ap()
    # w hi-parts duplicated per DR slot: [128, kt(3), s(2), 512]
    wqa = nc.dram_tensor("wqa", [128, 3, 2, INNER], E4, kind="ExternalInput").ap()
    wka = nc.dram_tensor("wka", [128, 3, 2, INNER], E4, kind="ExternalInput").ap()
    wva = nc.dram_tensor("wva", [128, 3, 2, INNER], E4, kind="ExternalInput").ap()
    # w lo-parts: [128, de(2), s(2), 512]; de=0 slots (lo-k0, lo-k1),
    # de=1 slots (ZERO, lo-k2)  — pairs with x slots (hi-k0,hi-k1)/(hi-k1,hi-k2)
    wqd = nc.dram_tensor("wqd", [128, 2, 2, INNER], E4, kind="ExternalInput").ap()
    wkd = nc.dram_tensor("wkd", [128, 2, 2, INNER], E4, kind="ExternalInput").ap()
    wvd = nc.dram_tensor("wvd", [128, 2, 2, INNER], E4, kind="ExternalInput").ap()
    # w_out rows permuted to HSLOT head order: [512, 384]
    wo = nc.dram_tensor("wo", [INNER, D], BF16, kind="ExternalInput").ap()
    # zero-diagonal band mask [128, 4, 128]
    mask = nc.dram_tensor("mask01", [128, 4, 128], BF16, kind="ExternalInput").ap()
    bias = nc.dram_tensor("bias", [128, D], F32, kind="ExternalInput").ap()
    brow = nc.dram_tensor("brow", [1, D], BF16, kind="ExternalInput").ap()
    ones = nc.dram_tensor("ones", [1, 128], BF16, kind="ExternalInput").ap()
    iddr = nc.dram_tensor("ident", [128, 128], BF16, kind="ExternalInput").ap()
    out = nc.dram_tensor("out", [T, D], F32, kind="ExternalOutput").ap()

    with tile.TileContext(nc) as tc, ExitStack() as ctx:
        const = ctx.enter_context(tc.tile_pool(name="const", bufs=1))

        xdr_sb = [
            const.tile([128, 3, 2, 512], E4, name=f"xdr{p}") for p in range(4)
        ]
        wqa_sb = const.tile([128, 3, 2, INNER], E4)
        wka_sb = const.tile([128, 3, 2, INNER], E4)
        wva_sb = const.tile([128, 3, 2, INNER], E4)
        wqd_sb = const.tile([128, 2, 2, INNER], E4)
        wkd_sb = const.tile([128, 2, 2, INNER], E4)
        wvd_sb = const.tile([128, 2, 2, INNER], E4)
        wo_sb = const.tile([128, 4, D], BF16)
        mk_sb = const.tile([128, 4, 128], BF16)
        bi_sb = const.tile([128, D], F32)
        br_sb = const.tile([1, D], BF16)
        on_sb = const.tile([1, 128], BF16)
        ident = const.tile([128, 128], BF16)

        # SBUF pools
        qt_pool = ctx.enter_context(tc.tile_pool(name="qt", bufs=3))
        kt_pool = ctx.enter_context(tc.tile_pool(name="kt", bufs=3))
        vv_pool = ctx.enter_context(tc.tile_pool(name="vv", bufs=4))
        pt_pool = ctx.enter_context(tc.tile_pool(name="pt", bufs=6))
        rp_pool = ctx.enter_context(tc.tile_pool(name="rp", bufs=6))
        an_pool = ctx.enter_context(tc.tile_pool(name="an", bufs=6))
        at_pool = ctx.enter_context(tc.tile_pool(name="at", bufs=3))
        ob_pool = ctx.enter_context(tc.tile_pool(name="ob", bufs=4))

        # PSUM: big pool (2-bank tiles, 3 bufs = 6 banks) hosts q/k-proj
        # double-groups [128,2,512], score tiles [128,4,256] and natural-
        # attention tiles [128,8,128]; small pool (1-bank, 2 bufs) hosts
        # v-proj [128,512] and out-proj [128,384] tiles.  6 + 2 = 8 banks.
        pB = ctx.enter_context(tc.tile_pool(name="pB", bufs=3, space="PSUM"))
        pS = ctx.enter_context(tc.tile_pool(name="pS", bufs=2, space="PSUM"))

        state: dict = {}

        def load_inputs(rep):
            # startup DMAs spread over all four DGE queues so issue overhead
            # and transfers overlap; each queue in consumption order
            # SP: q-weights + first-pair x chunks (split per kt)
            nc.sync.dma_start(out=wqa_sb[:], in_=wqa)
            nc.sync.dma_start(out=xdr_sb[0][:, 0], in_=xdr[0, :, 0])
            for bp in range(1, 4):
                nc.sync.dma_start(out=xdr_sb[bp][:], in_=xdr[bp])
            # Activation queue: the kt1 x-chunk in parallel with the SP
            # queue, then q lo-weights (group 0's 3rd matmul), k-weights
            nc.scalar.dma_start(out=xdr_sb[0][:, 1], in_=xdr[0, :, 1])
            nc.scalar.dma_start(out=xdr_sb[0][:, 2], in_=xdr[0, :, 2])
            nc.scalar.dma_start(out=wqd_sb[:], in_=wqd)
            nc.scalar.dma_start(out=wka_sb[:], in_=wka)
            nc.scalar.dma_start(out=wkd_sb[:], in_=wkd)
            # Pool/SWDGE queue: everything else, in consumption order
            nc.gpsimd.dma_start(out=wva_sb[:], in_=wva)
            nc.gpsimd.dma_start(out=wvd_sb[:], in_=wvd)
            nc.gpsimd.dma_start(out=mk_sb[:], in_=mask)
            nc.gpsimd.dma_start(
                out=wo_sb[:], in_=wo.rearrange("(t p) n -> p t n", p=128)
            )
            nc.gpsimd.dma_start(out=bi_sb[:], in_=bias)
            nc.gpsimd.dma_start(out=br_sb[:], in_=brow)
            nc.gpsimd.dma_start(out=on_sb[:], in_=ones)
            nc.gpsimd.dma_start(out=ident[:], in_=iddr)

        def qk_group(pair, g, eng=None):
            """One ft-pair group of the q^T/k^T projection (g in 0..3)."""
            t_p = 2 * pair * N
            if g == 0:
                state[("qk", pair)] = (
                    qt_pool.tile([128, 4, 2 * N], BF16, tag="qt", name=f"qT{pair}"),
                    kt_pool.tile([128, 4, 2 * N], BF16, tag="kt", name=f"kT{pair}"),
                )
            qT, kT = state[("qk", pair)]
            # group order (q-ft01, k-ft01, q-ft23, k-ft23): the first two
            # unblock head quad 0 of both batches
            wa_sb, wd_sb, dstT = (
                (wqa_sb, wqd_sb, qT) if g % 2 == 0 else (wka_sb, wkd_sb, kT)
            )
            fp = g // 2
            xp = xdr_sb[pair]
            ps = pB.tile([128, 2, INNER], F32, tag="pB")
            for fi in range(2):
                ft = 2 * fp + fi
                fsl = slice(ft * 128, (ft + 1) * 128)
                # kt2-dependent matmuls last: covers the kt2 chunk DMA
                for mi in range(5):
                    if mi < 2:
                        lhsT, rhs = wa_sb[:, mi, :, fsl], xp[:, mi, :, 0 : 2 * N]
                    elif mi == 2:
                        lhsT, rhs = wd_sb[:, 0, :, fsl], xp[:, 0:2, 0, 0 : 2 * N]
                    elif mi == 3:
                        lhsT, rhs = wa_sb[:, 2, :, fsl], xp[:, 2, :, 0 : 2 * N]
                    else:
                        lhsT, rhs = wd_sb[:, 1, :, fsl], xp[:, 2, :, 0 : 2 * N]
                    nc.tensor.matmul(
                        ps[:, fi, : 2 * N],
                        lhsT=lhsT,
                        rhs=rhs,
                        start=(mi == 0),
                        stop=(mi == 4),
                        perf_mode=DR,
                    )
            # one evac per ft-pair, 2^-10 unscale folded in
            if eng is None:
                eng = "s"
            dst = dstT[:, 2 * fp : 2 * fp + 2, :]
            if eng == "s":
                nc.scalar.mul(dst, ps[:, :, : 2 * N], UNSCALE)
            else:
                nc.vector.tensor_scalar_mul(dst, ps[:, :, : 2 * N], UNSCALE)

        def qk_proj(pair):
            for g in range(4):
                qk_group(pair, g)

        def qk_single(pair, w, ft, eng):
            """Single-ft q/k projection group on the small PSUM pool — used
            in the prologue to spread PSUM recycling across 5 buffers."""
            t_p = 2 * pair * N
            qT, kT = state[("qk", pair)]
            wa_sb, wd_sb, dstT = (
                (wqa_sb, wqd_sb, qT) if w == 0 else (wka_sb, wkd_sb, kT)
            )
            fsl = slice(ft * 128, (ft + 1) * 128)
            xp = xdr_sb[pair]
            ps = pS.tile([128, 2 * N], F32, tag="pS", name=f"pqs{w}_{ft}")
            for mi in range(5):
                if mi < 2:
                    lhsT, rhs = wa_sb[:, mi, :, fsl], xp[:, mi, :, 0 : 2 * N]
                elif mi == 2:
                    lhsT, rhs = wd_sb[:, 0, :, fsl], xp[:, 0:2, 0, 0 : 2 * N]
                elif mi == 3:
                    lhsT, rhs = wa_sb[:, 2, :, fsl], xp[:, 2, :, 0 : 2 * N]
                else:
                    lhsT, rhs = wd_sb[:, 1, :, fsl], xp[:, 2, :, 0 : 2 * N]
                nc.tensor.matmul(
                    ps[:, :],
                    lhsT=lhsT,
                    rhs=rhs,
                    start=(mi == 0),
                    stop=(mi == 4),
                    perf_mode=DR,
                )
            if eng == "s":
                nc.scalar.mul(dstT[:, ft, :], ps[:, :], UNSCALE)
            else:
                nc.vector.tensor_scalar_mul(dstT[:, ft, :], ps[:, :], UNSCALE)

        def v_proj(b):
            """v natural (ones column per head) for batch b, split-fp8 DR."""
            xp = xdr_sb[b // 2]
            off = (b % 2) * N
            vv = vv_pool.tile([128, 2, H * 65], BF16, tag="vv")
            for jt, (j0, jsz) in enumerate(JTILES):
                jb = slice(off + j0, off + j0 + jsz)
                pv = pS.tile([128, INNER], F32, tag="pS")
                for mi in range(5):
                    if mi < 2:
                        lhsT, rhs = xp[:, mi, :, jb], wva_sb[:, mi, :, :]
                    elif mi == 2:
                        lhsT, rhs = xp[:, 0:2, 0, jb], wvd_sb[:, 0, :, :]
                    elif mi == 3:
                        lhsT, rhs = xp[:, 2, :, jb], wva_sb[:, 2, :, :]
                    else:
                        lhsT, rhs = xp[:, 2, :, jb], wvd_sb[:, 1, :, :]
                    nc.tensor.matmul(
                        pv[:jsz, :],
                        lhsT=lhsT,
                        rhs=rhs,
                        start=(mi == 0),
                        stop=(mi == 4),
                        perf_mode=DR,
                    )
                vj = vv[:jsz, jt].rearrange("p (h c) -> p h c", c=65)
                nc.gpsimd.memset(vj[:, :, 64:65], 1.0)
                pvh = pv[:jsz, :].rearrange("p (h c) -> p h c", c=64)
                nc.vector.tensor_scalar_mul(vj[:, :, 0:64], pvh, UNSCALE)
            state[("vv", b)] = vv

        def scores(b, qi, fine=False):
            """S^T + exp + band mask for head quad qi of batch b.

            fine=True runs exp/mask in 2-head slices so downstream attn@v
            matmuls unblock sooner (used for the drain batch)."""
            qT, kT = state[("qk", b // 2)]
            off = (b % 2) * N
            # slot order pairs same-base heads (h, h+2) so each PSUM
            # accumulation pair shares one stationary partition base
            quad = QUADS[qi]
            hseq = (quad[0], quad[2], quad[1], quad[3])
            pt = pt_pool.tile([128, 2, 4, N], BF16, tag="pt")
            state[("pt", b, qi)] = pt
            for jt, (j0, jsz) in enumerate(JTILES):
                pd = pB.tile([128, 4, 256], F32, tag="pB")
                for hh, h in enumerate(hseq):
                    po = (h % 2) * 64
                    nc.tensor.matmul(
                        pd[:jsz, hh, :N],
                        lhsT=kT[po : po + 64, h // 2, off + j0 : off + j0 + jsz],
                        rhs=qT[po : po + 64, h // 2, off : off + N],
                        start=(hh % 2 == 0),
                        stop=(hh % 2 == 1),
                    )
                if fine:
                    for hp in range(2):
                        nc.scalar.activation(
                            pt[:jsz, jt, 2 * hp : 2 * hp + 2],
                            pd[:jsz, 2 * hp : 2 * hp + 2, :N],
                            EXP,
                        )
                        nc.vector.tensor_mul(
                            pt[:jsz, jt, 2 * hp : 2 * hp + 2, j0 : j0 + jsz],
                            pt[:jsz, jt, 2 * hp : 2 * hp + 2, j0 : j0 + jsz],
                            mk_sb[:jsz, 2 * hp : 2 * hp + 2, :jsz],
                        )
                else:
                    nc.scalar.activation(pt[:jsz, jt], pd[:jsz, :, :N], EXP)
                    # zero the self-token diagonal, band only (Pool: SBUF-
                    # only op, keeps the loaded DVE queue clear)
                    nc.gpsimd.tensor_mul(
                        pt[:jsz, jt, :, j0 : j0 + jsz],
                        pt[:jsz, jt, :, j0 : j0 + jsz],
                        mk_sb[:jsz, :, :jsz],
                    )

        def attnv(b, it, xbar=True, norm_pool=False):
            """attn @ [v|1] natural for i-tile `it`; normalize; transpose."""
            vv = state[("vv", b)]
            pts = (state[("pt", b, 0)], state[("pt", b, 1)])
            i0, isz = JTILES[it]
            pa = pB.tile([128, 8, 128], F32, tag="pB")
            for bank in range(2):
                for si in range(4):
                    h = 4 * bank + si
                    ptq = pts[h // 4]
                    slot = (0, 2, 1, 3)[h % 4]
                    for jt, (j0, jsz) in enumerate(JTILES):
                        nc.tensor.matmul(
                            pa[:isz, h, 0:65],
                            lhsT=ptq[:jsz, jt, slot, i0 : i0 + isz],
                            rhs=vv[:jsz, jt, h * 65 : h * 65 + 65],
                            start=(si == 0 and jt == 0),
                            stop=(si == 3 and jt == 1),
                        )
            rp = rp_pool.tile([128, 8], F32, tag="rp")
            nc.vector.reciprocal(rp[:isz, :].unsqueeze(2), pa[:isz, :, 64:65])
            an = an_pool.tile([128, INNER], BF16, tag="an")
            if it == 1 and xbar:
                # pad rows 69..80 for the XBAR transpose (base partition must
                # be 0/32/64/96; rows 64:69 are overwritten by the multiply)
                nc.gpsimd.memset(an[64:80, :], 0.0)
            norm = nc.gpsimd.tensor_mul if norm_pool else nc.vector.tensor_mul
            norm(
                an[:isz].rearrange("p (s c) -> p s c", c=64),
                pa[:isz, :, 0:64],
                rp[:isz, :].unsqueeze(2).broadcast_to([isz, 8, 64]),
            )
            if it == 0:
                aT = at_pool.tile([128, 4, ATW], BF16, tag="at")
                state[("at", b)] = aT
            else:
                aT = state[("at", b)]
            state[("an", b, it)] = an
            if xbar:
                if it == 0:
                    nc.sync.dma_start_transpose(
                        out=aT[:, :, 0:128], in_=an[0:128, :]
                    )
                else:
                    nc.sync.dma_start_transpose(
                        out=aT[:, :, 128:208], in_=an[0:80, :]
                    )

        def pe_tr(b, it):
            """PE-array transpose of an(b, it) into aT — for the drain batch,
            where the XBAR round-trip latency would sit on the critical path."""
            an = state.pop(("an", b, it))
            aT = state[("at", b)]
            rows = JTILES[it][1]
            rp_ = rows + rows % 2  # keep 4-byte slot alignment for PSUM
            psT = pS.tile([128, 4, rp_], BF16, tag="pS", name=f"psT{b}_{it}")
            for kb in range(4):
                nc.tensor.transpose(
                    psT[:, kb, :rows],
                    an[0:rows, kb * 128 : (kb + 1) * 128],
                    ident[:rows, :rows],
                )
            if it == 0:
                nc.vector.tensor_copy(aT[:, :, 0:128], psT[:, :, :rows])
            else:
                nc.scalar.copy(aT[:, :, 128 : 128 + rows], psT[:, :, :rows])

        def out_tile(b, it, tail=False, ceng="s"):
            """output projection + bias for batch b, token tile `it`.

            tail=True folds the bias in as a ones-row matmul (PE is idle in
            the drain) and evacuates with a Scalar copy, keeping the slow
            engines off the critical tail chain."""
            aT = state[("at", b)]
            t_b = b * N
            j0, jsz = JTILES[it]
            bias_mm = tail
            po_ = pS.tile([128, D], F32, tag="pS")
            for kf in range(4):
                nc.tensor.matmul(
                    po_[:jsz, :],
                    lhsT=aT[:, kf, j0 : j0 + jsz],
                    rhs=wo_sb[:, kf, :],
                    start=(kf == 0),
                    stop=(kf == 3) if not bias_mm else False,
                )
            ob = ob_pool.tile([128, D], F32, tag="ob")
            if bias_mm:
                # bias as a ones-row rank-1 matmul; evac is then a plain
                # copy, which the Scalar engine can take (it cannot add)
                nc.tensor.matmul(
                    po_[:jsz, :],
                    lhsT=on_sb[:1, :jsz],
                    rhs=br_sb[:1, :],
                    start=False,
                    stop=True,
                )
                if ceng == "s":
                    nc.scalar.copy(ob[:jsz, :], po_[:jsz, :])
                else:
                    nc.vector.tensor_copy(ob[:jsz, :], po_[:jsz, :])
            else:
                nc.vector.tensor_add(ob[:jsz, :], po_[:jsz, :], bi_sb[:jsz, :])
            nc.sync.dma_start(
                out=out[t_b + j0 : t_b + j0 + jsz, :], in_=ob[:jsz, :]
            )

        def out_proj(b, tail=False):
            out_tile(b, 0, tail=tail)
            out_tile(b, 1, tail=tail)

        for _rep in range(reps):
            if _rep == 0:
                load_inputs(_rep)
            # software-pipelined batch loop: qk groups interleave with scores
            # on odd batches; out_proj fills the exp/mask shadow on even ones
            # prologue: 5 PSUM buffers + 3 evac engines kill recycle stalls
            qk_group(0, 0, eng="s")
            qk_group(0, 1, eng="v")
            qk_group(0, 2, eng="s")
            qk_single(0, 1, 2, "s")
            qk_single(0, 1, 3, "v")
            v_proj(0)
            v_proj(1)
            for b in range(B_CORE - 2):
                scores(b, 0)
                scores(b, 1)
                if b % 2 == 0 and b // 2 + 1 <= 3:
                    # next pair's projections fill the exp/mask shadow,
                    # interleaved against their own PSUM-recycle latency
                    qk_group(b // 2 + 1, 0)
                    qk_group(b // 2 + 1, 1)
                    if 1 <= b:
                        v_proj(b + 1)
                    qk_group(b // 2 + 1, 2)
                    attnv(b, 0)
                    qk_group(b // 2 + 1, 3)
                    out_proj(b - 1) if b >= 1 else None
                else:
                    v_proj(b + 1)
                    out_proj(b - 1)
                    attnv(b, 0)
                attnv(b, 1)
            # last two batches: drain-friendly — batch 7's first score quad
            # moves early so the serial exp queue finishes sooner; deferred
            # output projections feed the PE while the final chains settle;
            # PE transposes replace the XBAR for batch 7 (the XBAR round-trip
            # would sit on the now-empty critical path)
            b = B_CORE - 2
            scores(b, 0)
            scores(b, 1)
            v_proj(b + 1)
            scores(b + 1, 0)
            out_proj(b - 1)
            attnv(b, 0)
            scores(b + 1, 1)
            attnv(b, 1)
            b = B_CORE - 1
            attnv(b, 0, xbar=False)
            out_proj(b - 1, tail=True)
            attnv(b, 1, xbar=False)
            pe_tr(b, 0)
            pe_tr(b, 1)
            out_tile(b, 0, tail=True, ceng="v")
            out_tile(b, 1, tail=True)

    return nc


_CACHE: dict = {}


def get_compiled():
    if "nc" not in _CACHE:
        nc = build_nc()
        nc.compile()
        _CACHE["nc"] = nc
    return _CACHE["nc"]


def _split8(a, s):
    a = np.asarray(a, np.float32) * s
    hi = a.astype(NPE4)
    lo = (a - hi.astype(np.float32)).astype(NPE4)
    return hi, lo


def make_in_maps(x, w_qkv, scale, w_out, b_out):
    x = np.asarray(x, np.float32)
    w_qkv = np.asarray(w_qkv, np.float32)
    scale = np.asarray(scale, np.float32)
    w_out = np.asarray(w_out, np.float32)
    b_out = np.asarray(b_out, np.float32)

    # fold the per-head LSA scale into Wq (exact in real arithmetic)
    scale_rep = np.repeat(scale, DH)  # [512]
    wq = w_qkv[:, :INNER] * scale_rep[None, :]
    wk = w_qkv[:, INNER : 2 * INNER]
    wv = w_qkv[:, 2 * INNER :]

    def w_packs(w):
        hi, lo = _split8(w, SW)  # [384, 512] each
        hi = hi.reshape(3, 128, INNER)
        lo = lo.reshape(3, 128, INNER)
        wa = np.empty((128, 3, 2, INNER), NPE4)
        wa[:, :, 0, :] = hi.transpose(1, 0, 2)
        wa[:, :, 1, :] = hi.transpose(1, 0, 2)
        wd = np.zeros((128, 2, 2, INNER), NPE4)
        wd[:, 0, 0, :] = lo[0]
        wd[:, 0, 1, :] = lo[1]
        # de=1: lo-k2 duplicated — pairs with x (hi-k2, lo-k2) to add
        # (x_hi + x_lo)*w_lo-k2, using the otherwise-empty slot
        wd[:, 1, 0, :] = lo[2]
        wd[:, 1, 1, :] = lo[2]
        return wa, wd

    wqa, wqd = w_packs(wq)
    wka, wkd = w_packs(wk)
    wva, wvd = w_packs(wv)

    wo = w_out.astype(NPBF16)

    bias = np.ascontiguousarray(np.broadcast_to(b_out, (128, D))).astype(np.float32)

    # zero-diagonal band mask, replicated along the head-quad axis
    mk = np.ones((128, 128), np.float32)
    np.fill_diagonal(mk, 0.0)
    mk = np.repeat(mk[:, None, :], 4, axis=1).astype(NPBF16)

    xs = x.reshape(NCORES, B_CORE * N, D)
    in_maps = []
    for c in range(NCORES):
        hi, lo = _split8(xs[c].T, SX)  # [384, T]
        xdr = np.zeros((4, 128, 3, 2, 512), NPE4)
        hi = hi.reshape(3, 128, 4, 2 * N)
        lo = lo.reshape(3, 128, 4, 2 * N)
        xdr[:, :, :, 0, 0 : 2 * N] = hi.transpose(2, 1, 0, 3)
        xdr[:, :, :, 1, 0 : 2 * N] = lo.transpose(2, 1, 0, 3)
        in_maps.append(
            {
                "xdr": xdr,
                "wqa": wqa,
                "wqd": wqd,
                "wka": wka,
                "wkd": wkd,
                "wva": wva,
                "wvd": wvd,
                "wo": wo,
                "mask01": mk,
                "bias": bias,
                "brow": b_out.reshape(1, D).astype(NPBF16),
                "ones": np.ones((1, 128), NPBF16),
                "ident": np.eye(128, dtype=NPBF16),
            }
        )
    return in_maps


def run(x, w_qkv, scale, w_out, b_out, trace=False):
    """Run on the 8 NeuronCores; returns (full_output, BassKernelResults)."""
    in_maps = make_in_maps(x, w_qkv, scale, w_out, b_out)
    nc = get_compiled()
    res = run_bass_kernel_spmd(nc, in_maps, core_ids=list(range(NCORES)), trace=trace)
    outs = [res.results[c]["out"].reshape(B_CORE, N, D) for c in range(NCORES)]
    full = np.concatenate(outs, axis=0).astype(np.float32)
    return full, res


def kernel(x, w_qkv, scale, w_out, b_out):
    full, _ = run(x, w_qkv, scale, w_out, b_out, trace=False)
    return full
